# revision 13
# baseline (speedup 1.0000x reference)
"""Trainium2 Bass kernel for the Attention_layer problem.

Shapes: N=64, L=576, T=32, D=2048.
Reference math:
    f1e = relu(feature_1)                       # (N, L, D)   [output 0]
    f2e = relu(feature_2)                       # (N, T, D)
    s1  = f1e @ fc_w                            # (N, L)
    s2  = f2e @ fc_w                            # (N, T)
    att = softmax_L(s1[:,None,:] + s2[:,:,None] + b)   # (N, T, L) [output 2]
    f2_out = relu(f2e + att @ f1e)              # (N, T, D)   [output 1]

Key identity: softmax along L is invariant to the per-row constant
s2[n,t] + b, so att[n,t,:] == softmax(s1[n,:]) for every t.  fc_b and s2
never affect any output.  f_hat[n] = p[n] @ f1e[n] is one row, broadcast
across T.

Sharding: data-parallel over batch N across the 8 cores (8 batches each).
"""

import numpy as np

N, L, T, D = 64, 576, 32, 2048
NCORES = 8
NB = N // NCORES            # batches per core
CHS = [128, 128, 128, 128, 64]   # L = 4*128 + 64 partition chunks
NCH = len(CHS)
DC = D // 512               # 4 moving-dim chunks of 512

_CACHE: dict = {}


def _build_nc(stage=99):
    import concourse.bass_isa as bass_isa
    import concourse.tile as tile
    from concourse import bacc, mybir
    from contextlib import ExitStack

    f32 = mybir.dt.float32
    AF = mybir.ActivationFunctionType
    ALU = mybir.AluOpType

    nc = bacc.Bacc("TRN2", target_bir_lowering=False, debug=False)

    f1 = nc.dram_tensor("f1", [NB, L, D], f32, kind="ExternalInput").ap()
    f2 = nc.dram_tensor("f2", [NB, T, D], f32, kind="ExternalInput").ap()
    w_rep = nc.dram_tensor("w_rep", [128, D], f32, kind="ExternalInput").ap()
    id32 = nc.dram_tensor("id32", [T, T], f32, kind="ExternalInput").ap()
    id128 = nc.dram_tensor("id128", [128, 128], f32, kind="ExternalInput").ap()

    f1e_out = nc.dram_tensor("f1e_out", [NB, L, D], f32, kind="ExternalOutput").ap()
    f2_out = nc.dram_tensor("f2_out", [NB, T, D], f32, kind="ExternalOutput").ap()
    att_out = nc.dram_tensor("att_out", [NB, T, L], f32, kind="ExternalOutput").ap()

    with ExitStack() as ctx:
        tc = ctx.enter_context(tile.TileContext(nc))
        consts = ctx.enter_context(tc.tile_pool(name="consts", bufs=1))
        tpool = ctx.enter_context(tc.tile_pool(name="tpool", bufs=2 * NCH))
        scr = ctx.enter_context(tc.tile_pool(name="scr", bufs=2))
        f2pool = ctx.enter_context(tc.tile_pool(name="f2pool", bufs=2))
        fopool = ctx.enter_context(tc.tile_pool(name="fopool", bufs=2))
        attpool = ctx.enter_context(tc.tile_pool(name="attpool", bufs=2))
        small = ctx.enter_context(tc.tile_pool(name="small", bufs=2))
        ps_att = ctx.enter_context(tc.tile_pool(name="ps_att", bufs=2, space="PSUM"))
        ps_fo = ctx.enter_context(tc.tile_pool(name="ps_fo", bufs=4, space="PSUM"))

        w_t = consts.tile([128, D], f32)
        nc.sync.dma_start(out=w_t[:], in_=w_rep[:])
        id32_t = consts.tile([T, T], f32)
        nc.sync.dma_start(out=id32_t[:], in_=id32[:])
        id128_t = consts.tile([128, 128], f32)
        nc.sync.dma_start(out=id128_t[:], in_=id128[:])
        zeros_t = consts.tile([128, T], f32)
        nc.vector.memset(zeros_t[:], 0.0)

        for n in range(NB):
            # ---- load f1[n] as 5 partition chunks, f2[n] ----
            ts = []
            for c in range(NCH):
                pc = CHS[c]
                t = tpool.tile([pc, D], f32, tag="T")
                nc.sync.dma_start(out=t[:], in_=f1[n, 128 * c : 128 * c + pc, :])
                ts.append(t)
            f2t = f2pool.tile([T, D], f32, tag="F2")
            nc.sync.dma_start(out=f2t[:], in_=f2[n, :, :])

            # ---- relu in place (ACT), store f1e ----
            for c in range(NCH):
                nc.scalar.activation(ts[c][:], ts[c][:], AF.Relu)
                nc.sync.dma_start(
                    out=f1e_out[n, 128 * c : 128 * c + CHS[c], :], in_=ts[c][:]
                )
            nc.scalar.activation(f2t[:], f2t[:], AF.Relu)

            if stage < 1:
                continue
            # ---- s1 chunks via fused mul+row-reduce on DVE ----
            s1mat = small.tile([128, NCH], f32, tag="s1mat")
            # chunk 4 only fills partitions 0..63; pad rest so exp() ~ 0
            nc.vector.memset(s1mat[64:128, NCH - 1 : NCH], -30.0)
            for c in range(NCH):
                pc = CHS[c]
                sc = scr.tile([pc, D], f32, tag="scr")
                nc.vector.affine_mul_reduce(
                    out=sc[:],
                    accum_out=s1mat[0:pc, c : c + 1],
                    in0=ts[c][:],
                    in1=w_t[0:pc, :],
                    scale=1.0,
                    bias=0.0,
                )

            if stage < 2:
                continue
            # ---- softmax over all 576 values (no max-shift needed: |s1|<~4) ----
            emat = small.tile([128, NCH], f32, tag="emat")
            rowsum = small.tile([128, 1], f32, tag="rowsum")
            nc.scalar.activation(emat[:], s1mat[:], AF.Exp, accum_out=rowsum[:])
            den_col = small.tile([128, 1], f32, tag="den_col")
            nc.gpsimd.partition_all_reduce(
                den_col[:], rowsum[:], channels=128, reduce_op=bass_isa.ReduceOp.add
            )
            rden_col = small.tile([128, 1], f32, tag="rden_col")
            nc.vector.reciprocal(rden_col[:], den_col[:])
            pmat = small.tile([128, NCH], f32, tag="pmat")
            nc.vector.tensor_scalar_mul(pmat[:], emat[:], rden_col[:])

            if stage < 3:
                continue
            # ---- replicate p along 32 free cols (lhsT for PE) ----
            preps = []
            for c in range(NCH):
                pc = CHS[c]
                pr = small.tile([pc, T], f32, tag=f"prep{c}")
                nc.vector.tensor_scalar_add(pr[:], zeros_t[0:pc, :], pmat[0:pc, c : c + 1])
                preps.append(pr)

            if stage < 4:
                continue
            # ---- att[n] = p broadcast across T rows (PE: p_rep.T @ I) ----
            att_ps = ps_att.tile([T, L], f32, tag="attps")
            for c in range(NCH):
                pc = CHS[c]
                nc.tensor.matmul(
                    att_ps[:, 128 * c : 128 * c + pc],
                    preps[c][:],
                    id128_t[0:pc, 0:pc],
                    start=True,
                    stop=True,
                )
            att_t = attpool.tile([T, L], f32, tag="att")
            nc.scalar.copy(att_t[:], att_ps[:])
            nc.sync.dma_start(out=att_out[n, :, :], in_=att_t[:])

            if stage < 5:
                continue
            # ---- f2_out = relu(f2e + p @ f1e), accumulated in PSUM ----
            fo_t = fopool.tile([T, D], f32, tag="fo")
            for dc in range(DC):
                sl = slice(512 * dc, 512 * (dc + 1))
                fo_ps = ps_fo.tile([T, 512], f32, tag="fops")
                nc.tensor.matmul(
                    fo_ps[:], id32_t[:], f2t[:, sl], start=True, stop=False
                )
                for c in range(NCH):
                    nc.tensor.matmul(
                        fo_ps[:],
                        preps[c][:],
                        ts[c][:, sl],
                        start=False,
                        stop=(c == NCH - 1),
                    )
                nc.scalar.activation(fo_t[:, sl], fo_ps[:], AF.Relu)
            nc.sync.dma_start(out=f2_out[n, :, :], in_=fo_t[:])

    nc.compile()
    return nc


def _get_nc():
    if "nc" not in _CACHE:
        _CACHE["nc"] = _build_nc()
    return _CACHE["nc"]


def _const_inputs(fc_w):
    return {
        "w_rep": np.ascontiguousarray(
            np.broadcast_to(fc_w.astype(np.float32), (128, D))
        ),
        "id32": np.eye(T, dtype=np.float32),
        "id128": np.eye(128, dtype=np.float32),
    }


def _in_maps(feature_1, feature_2, fc_w):
    consts = _const_inputs(np.asarray(fc_w))
    f1 = np.ascontiguousarray(np.asarray(feature_1, dtype=np.float32))
    f2 = np.ascontiguousarray(np.asarray(feature_2, dtype=np.float32))
    in_maps = []
    for i in range(NCORES):
        s = slice(i * NB, (i + 1) * NB)
        in_maps.append({"f1": f1[s], "f2": f2[s], **consts})
    return in_maps


def _gather(results):
    f1e = np.concatenate([r["f1e_out"] for r in results], axis=0)
    f2o = np.concatenate([r["f2_out"] for r in results], axis=0)
    att = np.concatenate([r["att_out"] for r in results], axis=0)
    return f1e, f2o, att


def run(feature_1, feature_2, fc_w, fc_b, trace=False):
    from concourse.bass_utils import run_bass_kernel_spmd

    nc = _get_nc()
    in_maps = _in_maps(feature_1, feature_2, fc_w)
    res = run_bass_kernel_spmd(nc, in_maps, list(range(NCORES)), trace=trace)
    return _gather(res.results), res.exec_time_ns


def kernel(feature_1, feature_2, fc_w, fc_b):
    outs, _ = run(feature_1, feature_2, fc_w, fc_b, trace=False)
    return outs


# revision 29
# speedup vs baseline: 7.1237x; 7.1237x over previous
"""Trainium2 Bass kernel for the Attention_layer problem.

Shapes: N=64, L=576, T=32, D=2048.
Reference math:
    f1e = relu(feature_1)                       # (N, L, D)   [output 0]
    f2e = relu(feature_2)                       # (N, T, D)
    s1  = f1e @ fc_w                            # (N, L)
    s2  = f2e @ fc_w                            # (N, T)
    att = softmax_L(s1[:,None,:] + s2[:,:,None] + b)   # (N, T, L) [output 2]
    f2_out = relu(f2e + att @ f1e)              # (N, T, D)   [output 1]

Key identity: softmax along L is invariant to the per-row constant
s2[n,t] + b, so att[n,t,:] == softmax(s1[n,:]) for every t.  fc_b and s2
never affect any output.  f_hat[n] = p[n] @ f1e[n] is one row, broadcast
across T.

Sharding: data-parallel over batch N across the 8 cores (8 batches each).
"""

import numpy as np

N, L, T, D = 64, 576, 32, 2048
NCORES = 8
NB = N // NCORES            # batches per core
CHS = [128, 128, 128, 128, 64]   # L = 4*128 + 64 partition chunks
NCH = len(CHS)
DC = D // 512               # 4 moving-dim chunks of 512

_CACHE: dict = {}


def _build_nc(stage=99, reps=1, tbufs=2, wbcast=True, split=1):
    import concourse.bass_isa as bass_isa
    import concourse.tile as tile
    from concourse import bacc, mybir
    from contextlib import ExitStack

    f32 = mybir.dt.float32
    AF = mybir.ActivationFunctionType
    ALU = mybir.AluOpType

    nc = bacc.Bacc("TRN2", target_bir_lowering=False, debug=False)

    f1 = nc.dram_tensor("f1", [NB, L, D], f32, kind="ExternalInput").ap()
    f2 = nc.dram_tensor("f2", [NB, T, D], f32, kind="ExternalInput").ap()
    if wbcast:
        w_in = nc.dram_tensor("w_row", [1, D], f32, kind="ExternalInput").ap()
    else:
        w_in = nc.dram_tensor("w_rep", [128, D], f32, kind="ExternalInput").ap()
    id32 = nc.dram_tensor("id32", [T, T], f32, kind="ExternalInput").ap()
    id128 = nc.dram_tensor("id128", [128, 128], f32, kind="ExternalInput").ap()

    f1e_out = nc.dram_tensor("f1e_out", [NB, L, D], f32, kind="ExternalOutput").ap()
    f2_out = nc.dram_tensor("f2_out", [NB, T, D], f32, kind="ExternalOutput").ap()
    att_out = nc.dram_tensor("att_out", [NB, T, L], f32, kind="ExternalOutput").ap()

    with ExitStack() as ctx:
        tc = ctx.enter_context(tile.TileContext(nc))
        consts = ctx.enter_context(tc.tile_pool(name="consts", bufs=1))
        tpool = ctx.enter_context(tc.tile_pool(name="tpool", bufs=tbufs))
        scr = ctx.enter_context(tc.tile_pool(name="scr", bufs=2))
        f2pool = ctx.enter_context(tc.tile_pool(name="f2pool", bufs=2))
        fopool = ctx.enter_context(tc.tile_pool(name="fopool", bufs=2))
        attpool = ctx.enter_context(tc.tile_pool(name="attpool", bufs=2))
        small = ctx.enter_context(tc.tile_pool(name="small", bufs=2))
        ps_att = ctx.enter_context(tc.tile_pool(name="ps_att", bufs=2, space="PSUM"))
        ps_fo = ctx.enter_context(tc.tile_pool(name="ps_fo", bufs=4, space="PSUM"))

        w_t = consts.tile([128, D], f32)
        if wbcast:
            w_row_t = consts.tile([1, D], f32)
            nc.sync.dma_start(out=w_row_t[:], in_=w_in[:])
            nc.gpsimd.partition_broadcast(w_t[:], w_row_t[:])
        else:
            nc.sync.dma_start(out=w_t[:], in_=w_in[:])
        id32_t = consts.tile([T, T], f32)
        nc.sync.dma_start(out=id32_t[:], in_=id32[:])
        id128_t = consts.tile([128, 128], f32)
        nc.sync.dma_start(out=id128_t[:], in_=id128[:])
        zeros_t = consts.tile([128, T], f32)
        nc.vector.memset(zeros_t[:], 0.0)

        for n in [n for _ in range(reps) for n in range(NB)]:
            # ---- load f1[n]: rows 0..511 as one [128, 4*D] tile (l = c*128+p,
            # 4 chunks side by side in the free dim), plus the 64-row tail ----
            big = tpool.tile([128, 4 * D], f32, tag="T")
            f1_big = f1[n, 0:512, :].rearrange("(c p) d -> p c d", p=128)
            hw = 4 // split
            for h in range(split):
                nc.sync.dma_start(
                    out=big[:, h * hw * D : (h + 1) * hw * D],
                    in_=f1_big[:, h * hw : (h + 1) * hw, :],
                )
            t4 = tpool.tile([64, D], f32, tag="T4")
            nc.sync.dma_start(out=t4[:], in_=f1[n, 512:576, :])
            f2t = f2pool.tile([T, D], f32, tag="F2")
            nc.sync.dma_start(out=f2t[:], in_=f2[n, :, :])

            # chunk views: c=0..3 slices of `big`, c=4 is t4
            def chunk(c, lo=0, hi=D):
                if c < 4:
                    return big[:, c * D + lo : c * D + hi]
                return t4[:, lo:hi]

            ts = [chunk(c) for c in range(NCH)]

            # ---- relu in place (ACT), store f1e ----
            nc.scalar.activation(big[:], big[:], AF.Relu)
            nc.scalar.activation(t4[:], t4[:], AF.Relu)
            nc.scalar.dma_start(
                out=f1e_out[n, 0:512, :].rearrange("(c p) d -> p c d", p=128),
                in_=big[:],
            )
            nc.scalar.dma_start(out=f1e_out[n, 512:576, :], in_=t4[:])
            nc.scalar.activation(f2t[:], f2t[:], AF.Relu)

            if stage < 1:
                continue
            # ---- s1 chunks via fused mul+row-reduce on DVE ----
            s1mat = small.tile([128, NCH], f32, tag="s1mat")
            # chunk 4 only fills partitions 0..63; pad rest so exp() ~ 0
            nc.vector.memset(s1mat[64:128, NCH - 1 : NCH], -30.0)
            for c in range(NCH):
                pc = CHS[c]
                sc = scr.tile([pc, D], f32, tag="scr")
                nc.vector.affine_mul_reduce(
                    out=sc[:],
                    accum_out=s1mat[0:pc, c : c + 1],
                    in0=ts[c],
                    in1=w_t[0:pc, :],
                    scale=1.0,
                    bias=0.0,
                )

            if stage < 2:
                continue
            # ---- softmax over all 576 values (no max-shift needed: |s1|<~4) ----
            emat = small.tile([128, NCH], f32, tag="emat")
            rowsum = small.tile([128, 1], f32, tag="rowsum")
            nc.scalar.activation(emat[:], s1mat[:], AF.Exp, accum_out=rowsum[:])
            den_col = small.tile([128, 1], f32, tag="den_col")
            nc.gpsimd.partition_all_reduce(
                den_col[:], rowsum[:], channels=128, reduce_op=bass_isa.ReduceOp.add
            )
            rden_col = small.tile([128, 1], f32, tag="rden_col")
            nc.vector.reciprocal(rden_col[:], den_col[:])
            pmat = small.tile([128, NCH], f32, tag="pmat")
            nc.vector.tensor_scalar_mul(pmat[:], emat[:], rden_col[:])

            if stage < 3:
                continue
            # ---- replicate p along 32 free cols (lhsT for PE) ----
            preps = []
            for c in range(NCH):
                pc = CHS[c]
                pr = small.tile([pc, T], f32, tag=f"prep{c}")
                nc.vector.tensor_scalar_add(pr[:], zeros_t[0:pc, :], pmat[0:pc, c : c + 1])
                preps.append(pr)

            if stage < 4:
                continue
            # ---- att[n] = p broadcast across T rows (PE: p_rep.T @ I) ----
            att_ps = ps_att.tile([T, L], f32, tag="attps")
            for c in range(NCH):
                pc = CHS[c]
                nc.tensor.matmul(
                    att_ps[:, 128 * c : 128 * c + pc],
                    preps[c][:],
                    id128_t[0:pc, 0:pc],
                    start=True,
                    stop=True,
                )
            att_t = attpool.tile([T, L], f32, tag="att")
            nc.scalar.copy(att_t[:], att_ps[:])
            nc.scalar.dma_start(out=att_out[n, :, :], in_=att_t[:])

            if stage < 5:
                continue
            # ---- f2_out = relu(f2e + p @ f1e), accumulated in PSUM ----
            fo_t = fopool.tile([T, D], f32, tag="fo")
            for dc in range(DC):
                sl = slice(512 * dc, 512 * (dc + 1))
                fo_ps = ps_fo.tile([T, 512], f32, tag="fops")
                nc.tensor.matmul(
                    fo_ps[:], id32_t[:], f2t[:, sl], start=True, stop=False
                )
                for c in range(NCH):
                    nc.tensor.matmul(
                        fo_ps[:],
                        preps[c][:],
                        chunk(c, 512 * dc, 512 * (dc + 1)),
                        start=False,
                        stop=(c == NCH - 1),
                    )
                nc.scalar.activation(fo_t[:, sl], fo_ps[:], AF.Relu)
            nc.scalar.dma_start(out=f2_out[n, :, :], in_=fo_t[:])

    nc.compile()
    return nc


def _get_nc():
    if "nc" not in _CACHE:
        _CACHE["nc"] = _build_nc()
    return _CACHE["nc"]


def _const_inputs(fc_w):
    return {
        "w_row": np.ascontiguousarray(fc_w.astype(np.float32).reshape(1, D)),
        "id32": np.eye(T, dtype=np.float32),
        "id128": np.eye(128, dtype=np.float32),
    }


def _in_maps(feature_1, feature_2, fc_w):
    consts = _const_inputs(np.asarray(fc_w))
    f1 = np.ascontiguousarray(np.asarray(feature_1, dtype=np.float32))
    f2 = np.ascontiguousarray(np.asarray(feature_2, dtype=np.float32))
    in_maps = []
    for i in range(NCORES):
        s = slice(i * NB, (i + 1) * NB)
        in_maps.append({"f1": f1[s], "f2": f2[s], **consts})
    return in_maps


def _gather(results):
    f1e = np.concatenate([r["f1e_out"] for r in results], axis=0)
    f2o = np.concatenate([r["f2_out"] for r in results], axis=0)
    att = np.concatenate([r["att_out"] for r in results], axis=0)
    return f1e, f2o, att


def run(feature_1, feature_2, fc_w, fc_b, trace=False):
    from concourse.bass_utils import run_bass_kernel_spmd

    nc = _get_nc()
    in_maps = _in_maps(feature_1, feature_2, fc_w)
    res = run_bass_kernel_spmd(nc, in_maps, list(range(NCORES)), trace=trace)
    return _gather(res.results), res.exec_time_ns


def kernel(feature_1, feature_2, fc_w, fc_b):
    outs, _ = run(feature_1, feature_2, fc_w, fc_b, trace=False)
    return outs


# revision 30
# speedup vs baseline: 10.8875x; 1.5284x over previous
"""Trainium2 Bass kernel for the Attention_layer problem.

Shapes: N=64, L=576, T=32, D=2048.
Reference math:
    f1e = relu(feature_1)                       # (N, L, D)   [output 0]
    f2e = relu(feature_2)                       # (N, T, D)
    s1  = f1e @ fc_w                            # (N, L)
    s2  = f2e @ fc_w                            # (N, T)
    att = softmax_L(s1[:,None,:] + s2[:,:,None] + b)   # (N, T, L) [output 2]
    f2_out = relu(f2e + att @ f1e)              # (N, T, D)   [output 1]

Key identity: softmax along L is invariant to the per-row constant
s2[n,t] + b, so att[n,t,:] == softmax(s1[n,:]) for every t.  fc_b and s2
never affect any output.  f_hat[n] = p[n] @ f1e[n] is one row, broadcast
across T.

Sharding: data-parallel over batch N across the 8 cores (8 batches each).
"""

import numpy as np

N, L, T, D = 64, 576, 32, 2048
NCORES = 8
NB = N // NCORES            # batches per core
CHS = [128, 128, 128, 128, 64]   # L = 4*128 + 64 partition chunks
NCH = len(CHS)
DC = D // 512               # 4 moving-dim chunks of 512

_CACHE: dict = {}


def _build_nc(stage=99, reps=1, tbufs=2, wbcast=True, split=1):
    import concourse.bass_isa as bass_isa
    import concourse.tile as tile
    from concourse import bacc, mybir
    from contextlib import ExitStack

    f32 = mybir.dt.float32
    AF = mybir.ActivationFunctionType
    ALU = mybir.AluOpType

    nc = bacc.Bacc("TRN2", target_bir_lowering=False, debug=False)

    f1 = nc.dram_tensor("f1", [NB, L, D], f32, kind="ExternalInput").ap()
    f2 = nc.dram_tensor("f2", [NB, T, D], f32, kind="ExternalInput").ap()
    if wbcast:
        w_in = nc.dram_tensor("w_row", [1, D], f32, kind="ExternalInput").ap()
    else:
        w_in = nc.dram_tensor("w_rep", [128, D], f32, kind="ExternalInput").ap()
    id32 = nc.dram_tensor("id32", [T, T], f32, kind="ExternalInput").ap()
    id128 = nc.dram_tensor("id128", [128, 128], f32, kind="ExternalInput").ap()

    f1e_out = nc.dram_tensor("f1e_out", [NB, L, D], f32, kind="ExternalOutput").ap()
    f2_out = nc.dram_tensor("f2_out", [NB, T, D], f32, kind="ExternalOutput").ap()
    att_out = nc.dram_tensor("att_out", [NB, T, L], f32, kind="ExternalOutput").ap()

    with ExitStack() as ctx:
        tc = ctx.enter_context(tile.TileContext(nc))
        consts = ctx.enter_context(tc.tile_pool(name="consts", bufs=1))
        tpool = ctx.enter_context(tc.tile_pool(name="tpool", bufs=tbufs))
        scr = ctx.enter_context(tc.tile_pool(name="scr", bufs=2))
        f2pool = ctx.enter_context(tc.tile_pool(name="f2pool", bufs=2))
        fopool = ctx.enter_context(tc.tile_pool(name="fopool", bufs=2))
        attpool = ctx.enter_context(tc.tile_pool(name="attpool", bufs=2))
        small = ctx.enter_context(tc.tile_pool(name="small", bufs=2))
        ps_att = ctx.enter_context(tc.tile_pool(name="ps_att", bufs=2, space="PSUM"))
        ps_fo = ctx.enter_context(tc.tile_pool(name="ps_fo", bufs=4, space="PSUM"))

        w_t = consts.tile([128, D], f32)
        if wbcast:
            w_row_t = consts.tile([1, D], f32)
            nc.sync.dma_start(out=w_row_t[:], in_=w_in[:])
            nc.gpsimd.partition_broadcast(w_t[:], w_row_t[:])
        else:
            nc.sync.dma_start(out=w_t[:], in_=w_in[:])
        id32_t = consts.tile([T, T], f32)
        nc.sync.dma_start(out=id32_t[:], in_=id32[:])
        id128_t = consts.tile([128, 128], f32)
        nc.sync.dma_start(out=id128_t[:], in_=id128[:])
        zeros_t = consts.tile([128, T], f32)
        nc.vector.memset(zeros_t[:], 0.0)

        for n in [n for _ in range(reps) for n in range(NB)]:
            # ---- load f1[n]: rows 0..511 as one [128, 4*D] tile (l = c*128+p,
            # 4 chunks side by side in the free dim), plus the 64-row tail ----
            big = tpool.tile([128, 4 * D], f32, tag="T")
            f1_big = f1[n, 0:512, :].rearrange("(c p) d -> p c d", p=128)
            hw = 4 // split
            for h in range(split):
                nc.sync.dma_start(
                    out=big[:, h * hw * D : (h + 1) * hw * D],
                    in_=f1_big[:, h * hw : (h + 1) * hw, :],
                )
            t4 = tpool.tile([64, D], f32, tag="T4")
            nc.sync.dma_start(out=t4[:], in_=f1[n, 512:576, :])
            f2t = f2pool.tile([T, D], f32, tag="F2")
            nc.sync.dma_start(out=f2t[:], in_=f2[n, :, :])

            # chunk views: c=0..3 slices of `big`, c=4 is t4
            def chunk(c, lo=0, hi=D):
                if c < 4:
                    return big[:, c * D + lo : c * D + hi]
                return t4[:, lo:hi]

            ts = [chunk(c) for c in range(NCH)]

            # ---- relu in place (ACT), store f1e ----
            nc.scalar.activation(big[:], big[:], AF.Relu)
            nc.scalar.activation(t4[:], t4[:], AF.Relu)
            nc.scalar.dma_start(
                out=f1e_out[n, 0:512, :].rearrange("(c p) d -> p c d", p=128),
                in_=big[:],
            )
            nc.scalar.dma_start(out=f1e_out[n, 512:576, :], in_=t4[:])
            nc.scalar.activation(f2t[:], f2t[:], AF.Relu)

            if stage < 1:
                continue
            # ---- s1 chunks via fused mul+row-reduce on DVE ----
            s1mat = small.tile([128, NCH], f32, tag="s1mat")
            # chunk 4 only fills partitions 0..63; pad rest so exp() ~ 0
            nc.vector.memset(s1mat[64:128, NCH - 1 : NCH], -30.0)
            for c in range(NCH):
                pc = CHS[c]
                sc = scr.tile([pc, D], f32, tag="scr")
                nc.vector.affine_mul_reduce(
                    out=sc[:],
                    accum_out=s1mat[0:pc, c : c + 1],
                    in0=ts[c],
                    in1=w_t[0:pc, :],
                    scale=1.0,
                    bias=0.0,
                )

            if stage < 2:
                continue
            # ---- softmax over all 576 values (no max-shift needed: |s1|<~4) ----
            emat = small.tile([128, NCH], f32, tag="emat")
            rowsum = small.tile([128, 1], f32, tag="rowsum")
            nc.scalar.activation(emat[:], s1mat[:], AF.Exp, accum_out=rowsum[:])
            den_col = small.tile([128, 1], f32, tag="den_col")
            nc.gpsimd.partition_all_reduce(
                den_col[:], rowsum[:], channels=128, reduce_op=bass_isa.ReduceOp.add
            )
            rden_col = small.tile([128, 1], f32, tag="rden_col")
            nc.vector.reciprocal(rden_col[:], den_col[:])
            pmat = small.tile([128, NCH], f32, tag="pmat")
            nc.vector.tensor_scalar_mul(pmat[:], emat[:], rden_col[:])

            if stage < 3:
                continue
            # ---- replicate p along 32 free cols (lhsT for PE) ----
            preps = []
            for c in range(NCH):
                pc = CHS[c]
                pr = small.tile([pc, T], f32, tag=f"prep{c}")
                nc.vector.tensor_scalar_add(pr[:], zeros_t[0:pc, :], pmat[0:pc, c : c + 1])
                preps.append(pr)

            if stage < 4:
                continue
            # ---- att[n] = p broadcast across T rows (PE: p_rep.T @ I) ----
            att_ps = ps_att.tile([T, L], f32, tag="attps")
            for c in range(NCH):
                pc = CHS[c]
                nc.tensor.matmul(
                    att_ps[:, 128 * c : 128 * c + pc],
                    preps[c][:],
                    id128_t[0:pc, 0:pc],
                    start=True,
                    stop=True,
                )
            att_t = attpool.tile([T, L], f32, tag="att")
            nc.scalar.copy(att_t[:], att_ps[:])
            nc.scalar.dma_start(out=att_out[n, :, :], in_=att_t[:])

            if stage < 5:
                continue
            # ---- f2_out = relu(f2e + p @ f1e), accumulated in PSUM ----
            fo_t = fopool.tile([T, D], f32, tag="fo")
            for dc in range(DC):
                sl = slice(512 * dc, 512 * (dc + 1))
                fo_ps = ps_fo.tile([T, 512], f32, tag="fops")
                nc.tensor.matmul(
                    fo_ps[:], id32_t[:], f2t[:, sl], start=True, stop=False
                )
                for c in range(NCH):
                    nc.tensor.matmul(
                        fo_ps[:],
                        preps[c][:],
                        chunk(c, 512 * dc, 512 * (dc + 1)),
                        start=False,
                        stop=(c == NCH - 1),
                    )
                nc.scalar.activation(fo_t[:, sl], fo_ps[:], AF.Relu)
            nc.scalar.dma_start(out=f2_out[n, :, :], in_=fo_t[:])

    nc.compile()
    return nc


def _get_nc():
    if "nc" not in _CACHE:
        _CACHE["nc"] = _build_nc()
    return _CACHE["nc"]


def _const_inputs(fc_w):
    return {
        "w_row": np.ascontiguousarray(fc_w.astype(np.float32).reshape(1, D)),
        "id32": np.eye(T, dtype=np.float32),
        "id128": np.eye(128, dtype=np.float32),
    }


def _in_maps(feature_1, feature_2, fc_w):
    consts = _const_inputs(np.asarray(fc_w))
    f1 = np.ascontiguousarray(np.asarray(feature_1, dtype=np.float32))
    f2 = np.ascontiguousarray(np.asarray(feature_2, dtype=np.float32))
    in_maps = []
    for i in range(NCORES):
        s = slice(i * NB, (i + 1) * NB)
        in_maps.append({"f1": f1[s], "f2": f2[s], **consts})
    return in_maps


def _gather(results):
    f1e = np.concatenate([r["f1e_out"] for r in results], axis=0)
    f2o = np.concatenate([r["f2_out"] for r in results], axis=0)
    att = np.concatenate([r["att_out"] for r in results], axis=0)
    return f1e, f2o, att


def run(feature_1, feature_2, fc_w, fc_b, trace=False):
    from concourse.bass_utils import run_bass_kernel_spmd

    nc = _get_nc()
    in_maps = _in_maps(feature_1, feature_2, fc_w)
    try:
        res = run_bass_kernel_spmd(nc, in_maps, list(range(NCORES)), trace=trace)
    except ModuleNotFoundError:
        # BASS_TRACE set but no NTFF profiling hook in this container —
        # rerun without tracing.
        import os

        os.environ["BASS_NEVER_TRACE"] = "1"
        res = run_bass_kernel_spmd(nc, in_maps, list(range(NCORES)), trace=False)
    return _gather(res.results), res.exec_time_ns


def kernel(feature_1, feature_2, fc_w, fc_b):
    outs, _ = run(feature_1, feature_2, fc_w, fc_b, trace=False)
    return outs
